# revision 1
# baseline (speedup 1.0000x reference)
"""Trainium2 Bass kernel for the EnhancedGNNEncoder (3-layer HydroConv GNN).

Strategy (8 NeuronCores, SPMD):
  - Nodes are range-partitioned across cores (dst-sharding): core c owns node
    rows [c*SLICE, (c+1)*SLICE). Each core aggregates messages for its own
    nodes only, computes the dense update (linear + relu + layernorm +
    residual) for its slice, and an AllGather rebuilds the full node table
    for the next layer's gathers.
  - Edges are routed to the core owning their dst. Host-side, edges are
    sorted by (src bucket, dst window). Per 128-edge tile, the device
    gathers h[src] rows with dma_gather (int16 indices, bucketed in 32768-row
    windows of the node table), scales by the per-edge weight w_e, and
    accumulates into per-dst-window PSUM tiles via a one-hot matmul
    (lhsT = messages [128e x 64f], rhs = one-hot [128e x 128nodes]).
  - The dst-gather of the reference (w * (h[src] - h[dst])) is eliminated
    algebraically: agg[n] = sum_e w_e h[src_e] - (sum_e w_e) h[n]. The
    second term is folded in as a per-node self-edge with weight
    -sum_e w_e (computed host-side from edge_attr, which does not depend
    on h).
  - Per-edge weights w_e = softplus(edge_attr @ emlp_w + emlp_b) depend only
    on inputs, so they are computed host-side and streamed per-edge.

The instruction stream is identical on all cores (SPMD); all per-core
variation lives in the input tensors (indices, weights, per-core x slice).
Per-(bucket,window) tile counts are max-reduced over cores and padded with
null edges (one-hot row of zeros).
"""

import math

import numpy as np

D = 64
L = 3
C = 8
WIN = 128
BUCKET = 32768
EPS = 1e-5
CH = 32  # gather-chunk size in 128-edge tiles

_CACHE = {}


def _softplus(z):
    return np.logaddexp(0.0, z)


def _prep(x, edge_index, edge_attr, lin_w, lin_b, emlp_w, emlp_b, gamma, beta,
          fc_w, fc_b):
    import ml_dtypes
    BF = ml_dtypes.bfloat16

    N = x.shape[0]
    E = edge_index.shape[1]
    NW = math.ceil(N / (C * WIN))
    SLICE = NW * WIN
    NPAD = C * SLICE
    NB = math.ceil(NPAD / BUCKET)

    src = np.ascontiguousarray(edge_index[0]).astype(np.int64)
    dst = np.ascontiguousarray(edge_index[1]).astype(np.int64)
    ea = np.asarray(edge_attr, dtype=np.float32)

    # per-layer edge weights + per-node weighted degree
    w_layers = np.empty((L, E), dtype=np.float32)
    wdeg = np.empty((L, NPAD), dtype=np.float32)
    for l in range(L):
        z = ea @ np.asarray(emlp_w[l, 0], dtype=np.float32) + float(emlp_b[l, 0])
        w_layers[l] = _softplus(z).astype(np.float32)
        wdeg[l] = np.bincount(dst, weights=w_layers[l].astype(np.float64),
                              minlength=NPAD).astype(np.float32)

    # append per-node self edges (weight -wdeg)
    selfn = np.arange(NPAD, dtype=np.int64)
    all_src = np.concatenate([src, selfn])
    all_dst = np.concatenate([dst, selfn])
    all_w = np.concatenate([w_layers, -wdeg], axis=1)  # [L, E+NPAD]

    core_of = all_dst // SLICE

    per_core = []
    counts = np.zeros((C, NB, NW), dtype=np.int64)
    for c in range(C):
        m = core_of == c
        s_c = all_src[m]
        d_c = all_dst[m]
        w_c = all_w[:, m]
        b_c = s_c // BUCKET
        wl_c = (d_c - c * SLICE) // WIN
        order = np.lexsort((wl_c, b_c))
        s_c, d_c, w_c = s_c[order], d_c[order], w_c[:, order]
        b_c, wl_c = b_c[order], wl_c[order]
        np.add.at(counts[c], (b_c, wl_c), 1)
        per_core.append((s_c, d_c, w_c, b_c, wl_c))

    maxcnt = counts.max(axis=0)  # [NB, NW]
    tiles = np.where(maxcnt > 0, (maxcnt + 127) // 128, 0).astype(np.int64)
    # group schedule shared across cores
    groups = []  # (bucket, window, n_tiles, tile_start)
    tpos = 0
    for b in range(NB):
        for w in range(NW):
            t = int(tiles[b, w])
            if t == 0:
                continue
            groups.append((b, w, t, tpos))
            tpos += t
    TOT_T = tpos
    TOT = TOT_T * 128

    # fill per-core streams
    idx16 = np.zeros((C, TOT), dtype=np.int16)
    dstloc = np.full((C, TOT), -1.0, dtype=np.float32)
    wvals = np.zeros((C, L, TOT), dtype=np.float32)
    for c in range(C):
        s_c, d_c, w_c, b_c, wl_c = per_core[c]
        # edges are sorted by (b, w); groups are in the same order
        epos = 0
        for (b, w, t, tstart) in groups:
            n = int(counts[c, b, w])
            if n:
                sl = slice(epos, epos + n)
                o = tstart * 128
                idx16[c, o:o + n] = (s_c[sl] - b * BUCKET).astype(np.int16)
                dstloc[c, o:o + n] = (d_c[sl] - (c * SLICE + w * WIN)).astype(np.float32)
                wvals[c, :, o:o + n] = w_c[:, sl]
                epos += n
        assert epos == len(s_c)

    # device layouts
    # wrapped gather indices: edge i -> [i % 16, i // 16], replicated x8
    idx_wrapped = np.zeros((C, 128, TOT // 16), dtype=np.int16)
    for c in range(C):
        w16 = idx16[c].reshape(TOT // 16, 16).T  # [16, TOT//16]
        idx_wrapped[c] = np.tile(w16, (8, 1))
    # per-tile-major: [128, TOT_T]: (p, t) = edge t*128+p
    dstloc_t = np.transpose(dstloc.reshape(C, TOT_T, 128), (0, 2, 1)).astype(BF)
    wvals_t = np.transpose(wvals.reshape(C, L, TOT_T, 128), (0, 1, 3, 2)).astype(BF)

    # chunks: consecutive tile runs within one bucket
    chunks = []  # (bucket, tile_start, n_tiles)
    for b in range(NB):
        bt = [g for g in groups if g[0] == b]
        if not bt:
            continue
        b0 = bt[0][3]
        bn = bt[-1][3] + bt[-1][2]
        t = b0
        while t < bn:
            ct = min(CH, bn - t)
            chunks.append((b, t, ct))
            t += ct

    # node table (padded) + per-core own slice in [128, NW, 64] layout
    x_pad = np.zeros((NPAD, D), dtype=np.float32)
    x_pad[:N] = np.asarray(x, dtype=np.float32)
    x_own = np.transpose(
        x_pad.reshape(C, NW, 128, D), (0, 2, 1, 3)).copy()  # [C, 128, NW, 64]

    iota = np.broadcast_to(np.arange(128, dtype=np.float32), (128, 1, 128)).astype(BF)
    id64 = np.eye(64, dtype=np.float32)
    id128 = np.eye(128, dtype=np.float32)
    lwT = np.transpose(np.asarray(lin_w, dtype=np.float32), (0, 2, 1)).astype(BF).copy()
    fwT = np.asarray(fc_w, dtype=np.float32).T.astype(BF).copy()

    gamma = np.asarray(gamma, dtype=np.float32)
    beta = np.asarray(beta, dtype=np.float32)
    ln_trivial = bool(np.all(gamma == 1.0) and np.all(beta == 0.0))

    meta = dict(N=N, NW=NW, SLICE=SLICE, NPAD=NPAD, NB=NB, TOT_T=TOT_T,
                groups=tuple(groups), chunks=tuple(chunks),
                ln_trivial=ln_trivial)

    in_maps = []
    for c in range(C):
        in_maps.append({
            "x_pad": x_pad,
            "x_own": x_own[c],
            "idx_w": idx_wrapped[c],
            "dstloc": dstloc_t[c],
            "wv": wvals_t[c],
            "iota": iota,
            "id64": id64,
            "id128": id128,
            "lwT": lwT,
            "lb": np.asarray(lin_b, dtype=np.float32),
            "fwT": fwT,
            "fb": np.asarray(fc_b, dtype=np.float32).reshape(64, 1),
            "gm": np.broadcast_to(gamma[:, None, :], (L, 128, D)).copy(),
            "bt": np.broadcast_to(beta[:, None, :], (L, 128, D)).copy(),
        })
    return meta, in_maps


def _split_multi_waits(nc, mybir):
    """This walrus build rejects >1 sync-wait per instruction; hoist extras
    onto single-wait NOPs inserted just before, same engine."""
    ctr = 0
    for bbw in nc.bb_map.values():
        bb = bbw.bb
        insts = bb.instructions
        new = []
        changed = False
        for inst in insts:
            si = inst.sync_info
            waits = list(si.on_wait) if si and si.on_wait else []
            if len(waits) > 1:
                changed = True
                for w in waits[:-1]:
                    ctr += 1
                    new.append(mybir.InstNoOp(
                        name=f"I-waitsplit-{ctr}",
                        engine=inst.engine,
                        sync_info=mybir.SyncInfo(on_wait=[w], on_update=[]),
                    ))
                si.on_wait = [waits[-1]]
            new.append(inst)
        if changed:
            bb.instructions = new


def _build(meta, split_waits=True, n_layers=L, do_fc=True, do_coll=True,
           do_agg=True, do_dense=True, do_ln=True):
    import concourse.bass as bass
    import concourse.mybir as mybir
    from concourse import library_config
    from concourse.library_overlay import lower_extended_insts
    from concourse.tile import TileContext

    NW = meta["NW"]
    SLICE = meta["SLICE"]
    NPAD = meta["NPAD"]
    NB = meta["NB"]
    TOT_T = meta["TOT_T"]
    groups = meta["groups"]
    chunks = meta["chunks"]
    ln_trivial = meta["ln_trivial"]
    TOT = TOT_T * 128

    F32 = mybir.dt.float32
    BF = mybir.dt.bfloat16
    I16 = mybir.dt.int16
    AF = mybir.ActivationFunctionType
    OP = mybir.AluOpType

    nc = bass.Bass(num_devices=C, num_swdge_queues=4)

    x_pad = nc.declare_dram_parameter("x_pad", [NPAD, D], F32, isOutput=False)
    x_own = nc.declare_dram_parameter("x_own", [128, NW, D], F32, isOutput=False)
    idx_w = nc.declare_dram_parameter("idx_w", [128, TOT // 16], I16, isOutput=False)
    dstloc = nc.declare_dram_parameter("dstloc", [128, TOT_T], BF, isOutput=False)
    wv = nc.declare_dram_parameter("wv", [L, 128, TOT_T], BF, isOutput=False)
    iota = nc.declare_dram_parameter("iota", [128, 1, 128], BF, isOutput=False)
    id64 = nc.declare_dram_parameter("id64", [64, 64], F32, isOutput=False)
    id128 = nc.declare_dram_parameter("id128", [128, 128], F32, isOutput=False)
    lwT = nc.declare_dram_parameter("lwT", [L, 64, 64], BF, isOutput=False)
    lb = nc.declare_dram_parameter("lb", [L, 64], F32, isOutput=False)
    fwT = nc.declare_dram_parameter("fwT", [64, 64], BF, isOutput=False)
    fb = nc.declare_dram_parameter("fb", [64, 1], F32, isOutput=False)
    if not ln_trivial:
        gm = nc.declare_dram_parameter("gm", [L, 128, 64], F32, isOutput=False)
        bt = nc.declare_dram_parameter("bt", [L, 128, 64], F32, isOutput=False)
    out = nc.declare_dram_parameter("out", [128, NW, D], F32, isOutput=True)

    if do_coll and n_layers > 1:
        tabs = [
            nc.dram_tensor("tabA", [NPAD, D], F32, addr_space="Shared"),
            nc.dram_tensor("tabB", [NPAD, D], F32, addr_space="Shared"),
        ]
        slice_outs = [
            nc.dram_tensor("slice0", [SLICE, D], F32),
            nc.dram_tensor("slice1", [SLICE, D], F32),
        ]
    else:
        tabs, slice_outs = [x_pad, x_pad], []

    nc.gpsimd.load_library(library_config.mlp)

    with TileContext(nc) as tc:
        with (
            tc.tile_pool(name="const", bufs=1) as cpool,
            tc.tile_pool(name="big", bufs=1) as bigp,
            tc.tile_pool(name="gat", bufs=6) as gpool,
            tc.tile_pool(name="msg", bufs=4) as mpool,
            tc.tile_pool(name="oh", bufs=4) as opool,
            tc.tile_pool(name="strm", bufs=6) as stp,
            tc.tile_pool(name="dense", bufs=3) as dpool,
            tc.tile_pool(name="psagg", bufs=4, space="PSUM") as ps_agg,
            tc.tile_pool(name="psd", bufs=2, space="PSUM") as ps_d,
            tc.tile_pool(name="pst", bufs=2, space="PSUM") as ps_t,
        ):
            # constants
            iota_t = cpool.tile([128, 1, 128], BF)
            nc.sync.dma_start(out=iota_t[:], in_=iota[:, :, :])
            id64_t = cpool.tile([64, 64], F32)
            nc.sync.dma_start(out=id64_t[:], in_=id64[:, :])
            id128_t = cpool.tile([128, 128], F32)
            nc.sync.dma_start(out=id128_t[:], in_=id128[:, :])
            lwT_ts = []
            for l in range(L):
                t = cpool.tile([64, 64], BF, tag=f"lwT{l}")
                nc.sync.dma_start(out=t[:], in_=lwT[l, :, :])
                lwT_ts.append(t)
            lb_ts = []
            for l in range(L):
                t = cpool.tile([64, 1], F32, tag=f"lb{l}")
                nc.sync.dma_start(out=t[:], in_=lb[l, :, None])
                lb_ts.append(t)
            fwT_t = cpool.tile([64, 64], BF)
            nc.sync.dma_start(out=fwT_t[:], in_=fwT[:, :])
            fb_t = cpool.tile([64, 1], F32)
            nc.sync.dma_start(out=fb_t[:], in_=fb[:, :])
            gm_ts, bt_ts = [], []
            if not ln_trivial:
                for l in range(L):
                    g_ = cpool.tile([128, 64], F32, tag=f"gm{l}")
                    nc.sync.dma_start(out=g_[:], in_=gm[l, :, :])
                    gm_ts.append(g_)
                    b_ = cpool.tile([128, 64], F32, tag=f"bt{l}")
                    nc.sync.dma_start(out=b_[:], in_=bt[l, :, :])
                    bt_ts.append(b_)

            eps_t = cpool.tile([128, 1], F32)
            nc.vector.memset(eps_t[:], EPS)
            # one register per distinct gather size, reused across all calls
            nidx_regs = {}
            for (_b, _t0, _ct) in chunks:
                v = _ct * 128
                if v not in nidx_regs:
                    nidx_regs[v] = nc.gpsimd.to_reg(v)

            own = [bigp.tile([128, NW, D], F32, tag="own_a", name="own_a"),
                   bigp.tile([128, NW, D], F32, tag="own_b", name="own_b")]
            nc.sync.dma_start(out=own[0][:], in_=x_own[:, :, :])
            agg = bigp.tile([64, NW * 128], BF, tag="agg", name="agg")

            for l in range(n_layers):
                tab_in = x_pad if l == 0 else tabs[l - 1]
                own_cur = own[l % 2]
                own_nxt = own[(l + 1) % 2]

                nc.vector.memset(agg[:], 0.0)

                open_ps = {}
                for ci_, (b, t0, ct) in enumerate(chunks if do_agg else []):
                    nidx = ct * 128
                    idx_t = stp.tile([128, ct * 8], I16, tag="idx", name="idx")
                    nc.sync.dma_start(out=idx_t[:],
                                      in_=idx_w[:, t0 * 8:(t0 + ct) * 8])
                    dst_t = stp.tile([128, ct], BF, tag="dst", name="dst")
                    nc.sync.dma_start(out=dst_t[:],
                                      in_=dstloc[:, t0:t0 + ct])
                    w_t = stp.tile([128, ct], BF, tag="w", name="w")
                    nc.sync.dma_start(out=w_t[:],
                                      in_=wv[l, :, t0:t0 + ct])

                    gat = gpool.tile([128, ct, D], F32, tag="gat", name="gat")
                    brows = min(BUCKET, NPAD - b * BUCKET)
                    nc.gpsimd.dma_gather(
                        out_ap=gat[:],
                        in_ap=tab_in[b * BUCKET:b * BUCKET + brows, :],
                        idxs_ap=idx_t[:],
                        num_idxs=nidx,
                        num_idxs_reg=nidx_regs[nidx],
                        elem_size=D,
                        single_packet=False,
                        queue_num=ci_ % 4,
                    )
                    msgs = mpool.tile([128, ct, D], BF, tag="msgs", name="msgs")
                    nc.scalar.copy(msgs[:], gat[:])
                    nc.vector.tensor_tensor(
                        out=msgs[:],
                        in0=msgs[:],
                        in1=w_t[:, :, None].to_broadcast([128, ct, D]),
                        op=OP.mult,
                    )
                    oh = opool.tile([128, ct, 128], BF, tag="oh", name="oh")
                    nc.vector.tensor_tensor(
                        out=oh[:],
                        in0=dst_t[:, :, None].to_broadcast([128, ct, 128]),
                        in1=iota_t[:].to_broadcast([128, ct, 128]),
                        op=OP.is_equal,
                    )
                    # matmuls per tile
                    for gi, (gb, gw, gt, gstart) in enumerate(groups):
                        if gb != b:
                            continue
                        lo = max(gstart, t0)
                        hi = min(gstart + gt, t0 + ct)
                        if lo >= hi:
                            continue
                        if gstart >= t0 and gstart < t0 + ct:
                            open_ps[gi] = ps_agg.tile([64, 128], F32, tag="psagg", name="psagg")
                        ps = open_ps[gi]
                        for t in range(lo, hi):
                            ti = t - t0
                            nc.tensor.matmul(
                                ps[:],
                                lhsT=msgs[:, ti, :],
                                rhs=oh[:, ti, :],
                                start=(t == gstart),
                                stop=(t == gstart + gt - 1),
                            )
                        if gstart + gt <= t0 + ct:
                            # group complete: flush into agg
                            nc.vector.tensor_tensor(
                                out=agg[:, gw * 128:(gw + 1) * 128],
                                in0=agg[:, gw * 128:(gw + 1) * 128],
                                in1=ps[:],
                                op=OP.add,
                            )
                            del open_ps[gi]
                assert not open_ps

                # dense phase per window
                for w in range(NW) if do_dense else []:
                    pd = ps_d.tile([64, 128], F32, tag="psd", name="psd")
                    nc.tensor.matmul(pd[:], lhsT=lwT_ts[l][:],
                                     rhs=agg[:, w * 128:(w + 1) * 128],
                                     start=True, stop=True)
                    rT = dpool.tile([64, 128], F32, tag="rT", name="rT")
                    nc.scalar.activation(rT[:], pd[:], AF.Relu,
                                         bias=lb_ts[l][:, 0:1])
                    pt = ps_t.tile([128, 64], F32, tag="pst", name="pst")
                    nc.tensor.transpose(pt[:], rT[:], id64_t[:])
                    nc.scalar.copy(own_nxt[:, w, :], pt[:])

                # batched layernorm + residual over own_nxt
                if not do_ln:
                    continue
                mu_s = dpool.tile([128, NW], F32, tag="mu", name="mu")
                nc.vector.tensor_reduce(mu_s[:], own_nxt[:],
                                        axis=mybir.AxisListType.X, op=OP.add)
                sq = bigp.tile([128, NW, D], BF, tag="sq", name="sq")
                nc.scalar.activation(sq[:], own_nxt[:], AF.Square)
                ssq = dpool.tile([128, NW], F32, tag="ssq", name="ssq")
                nc.vector.tensor_reduce(ssq[:], sq[:],
                                        axis=mybir.AxisListType.X, op=OP.add)
                a2 = dpool.tile([128, NW], F32, tag="a2", name="a2")
                nc.vector.tensor_tensor(out=a2[:], in0=mu_s[:], in1=mu_s[:],
                                        op=OP.mult)
                bvar = dpool.tile([128, NW], F32, tag="bvar", name="bvar")
                nc.vector.scalar_tensor_tensor(
                    out=bvar[:], in0=a2[:], scalar=-1.0 / D, in1=ssq[:],
                    op0=OP.mult, op1=OP.add)
                std = dpool.tile([128, NW], F32, tag="std", name="std")
                nc.scalar.activation(std[:], bvar[:], AF.Sqrt,
                                     bias=eps_t[:, 0:1], scale=1.0 / D)
                rstd = dpool.tile([128, NW], F32, tag="rstd", name="rstd")
                nc.vector.reciprocal(rstd[:], std[:])
                xc = bigp.tile([128, NW, D], BF, tag="sq", name="sq")  # reuse sq slot
                nc.vector.scalar_tensor_tensor(
                    out=xc[:], in0=mu_s[:, :, None].to_broadcast([128, NW, D]),
                    scalar=-1.0 / D, in1=own_nxt[:],
                    op0=OP.mult, op1=OP.add)
                nc.vector.tensor_tensor(
                    out=own_nxt[:], in0=xc[:],
                    in1=rstd[:, :, None].to_broadcast([128, NW, D]),
                    op=OP.mult)
                if not ln_trivial:
                    nc.vector.tensor_tensor(
                        out=own_nxt[:], in0=own_nxt[:],
                        in1=gm_ts[l][:, None, :].to_broadcast([128, NW, D]),
                        op=OP.mult)
                    nc.vector.tensor_tensor(
                        out=own_nxt[:], in0=own_nxt[:],
                        in1=bt_ts[l][:, None, :].to_broadcast([128, NW, D]),
                        op=OP.add)
                nc.vector.tensor_tensor(out=own_nxt[:], in0=own_nxt[:],
                                        in1=own_cur[:], op=OP.add)

                if l < n_layers - 1 and do_coll:
                    so = slice_outs[l]
                    so_ap = so.ap().rearrange("(w p) f -> p w f", p=128)
                    nc.sync.dma_start(out=so_ap, in_=own_nxt[:])
                    nc.gpsimd.collective_compute(
                        "AllGather",
                        mybir.AluOpType.bypass,
                        replica_groups=[list(range(C))],
                        ins=[so[:].opt()],
                        outs=[tabs[l][:].opt()],
                    )

            # final fc on own slice
            h_fin = own[n_layers % 2]
            stage = own[(n_layers + 1) % 2]
            for w in range(NW) if do_fc else []:
                pt = ps_t.tile([64, 128], F32, tag="pst", name="pst")
                nc.tensor.transpose(pt[:], h_fin[:, w, :], id128_t[:])
                hT = dpool.tile([64, 128], BF, tag="hT", name="hT")
                nc.scalar.copy(hT[:], pt[:])
                po = ps_d.tile([64, 128], F32, tag="psd", name="psd")
                nc.tensor.matmul(po[:], lhsT=fwT_t[:], rhs=hT[:],
                                 start=True, stop=True)
                ob = dpool.tile([64, 128], F32, tag="ob", name="ob")
                nc.vector.tensor_scalar_add(ob[:], po[:], fb_t[:, 0:1])
                pq = ps_t.tile([128, 64], F32, tag="pst", name="pst")
                nc.tensor.transpose(pq[:], ob[:], id64_t[:])
                nc.scalar.copy(stage[:, w, :], pq[:])
            nc.sync.dma_start(out=out[:, :, :], in_=stage[:])

    if split_waits:
        _split_multi_waits(nc, mybir)
    lower_extended_insts(nc)
    return nc


def kernel(**inputs):
    from concourse.bass_utils import run_bass_kernel_spmd

    x = np.asarray(inputs["x"])
    meta, in_maps = _prep(
        x, np.asarray(inputs["edge_index"]), np.asarray(inputs["edge_attr"]),
        np.asarray(inputs["lin_w"]), np.asarray(inputs["lin_b"]),
        np.asarray(inputs["emlp_w"]), np.asarray(inputs["emlp_b"]),
        np.asarray(inputs["gamma"]), np.asarray(inputs["beta"]),
        np.asarray(inputs["fc_w"]), np.asarray(inputs["fc_b"]))

    key = (meta["NW"], meta["TOT_T"], meta["groups"], meta["chunks"],
           meta["ln_trivial"])
    if key not in _CACHE:
        _CACHE[key] = _build(meta)
    nc = _CACHE[key]

    res = run_bass_kernel_spmd(nc, in_maps, list(range(C)))
    N = meta["N"]
    NW = meta["NW"]
    parts = []
    for c in range(C):
        o = np.asarray(res.results[c]["out"])  # [128, NW, 64]
        parts.append(np.transpose(o, (1, 0, 2)).reshape(NW * 128, D))
    full = np.concatenate(parts, axis=0)[:N]
    return full.astype(np.float32)

